# revision 45
# baseline (speedup 1.0000x reference)
"""BitLinear (RMSNorm + ternary-quantized matmul) TRN2 kernel, v2.

Computation (reference semantics):
    x_norm = x * rsqrt(mean(x^2, -1) + 1e-6) * gamma          [B,S,Din]
    scale  = max(mean(|weight|), 1e-5)                        scalar
    wq     = round(clip(weight/scale, -1, 1))  in {-1,0,1}    [Dout,Din]
    out    = (x_norm @ wq.T) * scale                          [B,S,Dout]

Distribution (8 cores, full inputs in / full output out): 2-D shard,
4 token-groups x 2 output-feature halves.  Core c handles tokens
[tg*2048,(tg+1)*2048) x out-features [j*4096,(j+1)*4096), tg=c//2, j=c%2.
x is staged host-transposed ([Din, T] slices) so no PE transposes are
needed on device; per-token sum(x^2) is computed with an all-ones
stationary matmul that leaves the result broadcast across partitions,
exactly the layout needed to scale x^T columns.

Contraction split (accuracy/speed trade): the first A16 of 16 k-tiles
(128 each) run as fp16 matmuls (1 col/cycle); the last 2*B8 k-tiles run
as fp8e4 DoubleRow matmuls (2 k-planes per instruction, 2 MACs/cell).
Ternary weights are exact in both dtypes; only the fp8 cast of x_norm
loses precision (measured L2 rel err 1.2e-2/1.46e-2/1.69e-2 for
B8=2/3/4 vs the 2e-2 gate; fp16-only is 7.7e-4).

Scale factoring keeps every fp8 operand in e4m3 normal range with an
exactly representable weight value:
    fp16 part: xnT = fp16(xn * gamma * tau), w = {-2,0,2}      (tau=s/2)
    fp8  part: xq8 = e4m3(xn * gamma * 16 s), w8 = {-2,0,2} * 2^-5
    both products equal xn * gamma * s * ternary, one PSUM group.

The global scale s = max(mean|w|, 1e-5) and the Sign-tie nudge are
computed on host (pure scalar preprocessing); output is staged fp16 on
device and assembled/upcast to fp32 on host.
"""

import os
import sys

sys.path.insert(0, "/opt/trn_rl_repo")

import numpy as np

N_CORES = 8
B, S, D_IN, D_OUT = 4, 2048, 2048, 8192
T = B * S                    # 8192 tokens
G, H = 4, 2                  # token groups x out-feature shards
TPC = T // G                 # 2048 tokens per core
OPC = D_OUT // H             # 4096 out features per core
P = 128
KO = D_IN // P               # 16 k-tiles of 128
A16 = int(os.environ.get("BASS_A16", "8"))   # fp16 k-tiles (rest run fp8-DR)
B8 = (KO - A16) // 2         # fp8 DoubleRow groups (2 k-tiles each)
TG = 512                     # tokens per RMSNorm group
NTG = TPC // TG              # 4
NT = TPC // P                # 16 token tiles per core
OB = 512                     # out-feature block (OB//512 PSUM banks)
NCH = OB // 512              # PSUM chunks per block
NOB = OPC // OB
QP = 2.0 ** -5               # fp8 weight magnitude quantum / 2
EPS_RMS = 1e-6
EPS_SCALE = 1e-5

_BUILT = {}
LAST_PROFILE = {}


def _legalize_waits(nc):
    """Split multi-wait sync_info into preceding single-wait NOPs.

    The walrus build in this container caps embedded sync waits at 1 per
    instruction (2 for EventSemaphore); Tile's kernel-tail drain exceeds it.
    """
    from concourse import mybir

    n_fixed = 0
    for bb in nc.main_func.blocks:
        out = []
        changed = False
        for inst in bb.instructions:
            si = inst.sync_info
            waits = list(si.on_wait) if si is not None and si.on_wait else []
            cap = 2 if isinstance(inst, mybir.InstEventSemaphore) else 1
            if len(waits) > cap:
                for w in waits[:-cap]:
                    out.append(
                        mybir.InstNoOp(
                            name=f"{inst.name}-ws{n_fixed}",
                            engine=inst.engine,
                            sync_info=mybir.SyncInfo(on_wait=[w], on_update=[]),
                            text_hint="waitsplit",
                            bass_nofuse=True,
                        )
                    )
                    n_fixed += 1
                si.on_wait = waits[-cap:]
                changed = True
            out.append(inst)
        if changed:
            bb.instructions = out
    return n_fixed


def _build_main_kernel(uniform_gamma=False):
    import concourse.bass as bass
    import concourse.tile as tile
    from concourse import mybir

    f32 = mybir.dt.float32
    fp16 = mybir.dt.float16
    fp8 = mybir.dt.float8e4
    AF = mybir.ActivationFunctionType
    ALU = mybir.AluOpType
    DR = mybir.MatmulPerfMode.DoubleRow

    nc = bass.Bass()
    x_in = nc.dram_tensor("x", [D_IN, TPC], f32, kind="ExternalInput")
    wt_in = nc.dram_tensor("wt", [D_IN, OPC], f32, kind="ExternalInput")
    g_in = nc.dram_tensor("gamma", [D_IN], f32, kind="ExternalInput")
    # scalars = [tau, tau_bias, c8]: tau = scale/2; tau_bias is tau possibly
    # nudged one ulp up by the host so no |w| bit-equals it (Sign(0) at an
    # exact tie would emit a half-quantum); c8 = 16*scale (fp8 x prescale).
    s_in = nc.dram_tensor("scalars", [3], f32, kind="ExternalInput")
    out = nc.dram_tensor("out", [TPC, OPC], fp16, kind="ExternalOutput")

    x3 = x_in.rearrange("(ko p) t -> p ko t", p=P)    # [128, 16, TPC]
    w3 = wt_in.rearrange("(ko p) o -> p ko o", p=P)   # [128, 16, OPC]

    with tile.TileContext(nc) as tc:
        with (
            tc.tile_pool(name="singles", bufs=1) as singles,
            tc.tile_pool(name="xt", bufs=18) as xtp,
            tc.tile_pool(name="xsq", bufs=3) as xsqp,
            tc.tile_pool(name="stats", bufs=2) as stats,
            tc.tile_pool(name="giv", bufs=3) as givp,
            tc.tile_pool(name="wraw", bufs=3) as wrawp,
            tc.tile_pool(name="wm", bufs=3) as wmp,
            tc.tile_pool(name="wq16", bufs=2) as wq16p,
            tc.tile_pool(name="wq8", bufs=2) as wq8p,
            tc.tile_pool(name="op", bufs=4) as op,
            tc.tile_pool(name="ssps", bufs=2, space="PSUM") as tps,
            tc.tile_pool(name="mps", bufs=4 // NCH, space="PSUM") as mps,
        ):
            # ---- constants ----
            ones_t = singles.tile([P, P], fp16)
            nc.vector.memset(ones_t[:], 1.0)
            eps_t = singles.tile([P, 1], f32)
            nc.vector.memset(eps_t[:], EPS_RMS)
            tau_sb = singles.tile([P, 1], f32)
            nc.sync.dma_start(tau_sb[:], s_in[0:1].to_broadcast((P, 1)))
            taub_sb = singles.tile([P, 1], f32)
            nc.sync.dma_start(taub_sb[:], s_in[1:2].to_broadcast((P, 1)))
            c8_sb = singles.tile([P, 1], f32)
            nc.sync.dma_start(c8_sb[:], s_in[2:3].to_broadcast((P, 1)))
            ntaub_sb = singles.tile([P, 1], f32)
            nc.vector.tensor_scalar_mul(ntaub_sb[:], taub_sb[:], -1.0)
            gamma_sb = singles.tile([P, KO], f32)
            nc.sync.dma_start(gamma_sb[:], g_in.rearrange("(ko p) -> p ko", p=P))
            # per-(k-partition) factors folded into x^T:
            #   fp16 tiles: gamma * tau ;  fp8 tiles: gamma * c8
            gs16 = singles.tile([P, KO], f32)
            nc.vector.tensor_scalar_mul(gs16[:], gamma_sb[:], tau_sb[:, 0:1])
            gs8 = singles.tile([P, KO], f32)
            nc.vector.tensor_scalar_mul(gs8[:], gamma_sb[:], c8_sb[:, 0:1])

            # x_norm^T resident for the whole kernel
            xnT16 = None
            if A16 > 0:
                xnT16 = singles.tile([P, A16, TPC], fp16, name="xnT16")
            xnT8 = [
                singles.tile([P, 2, TPC], fp8, name=f"xnT8_{g}") for g in range(B8)
            ]

            # ---- phase X: RMSNorm for tokens [ts0, ts0+glen) via ones-matmul ----
            def phase_x(ts0, glen):
                xts = []
                ps_ss = tps.tile([P, TG], f32, name="ps_ss")
                for ko in range(KO):
                    xt = xtp.tile([P, TG], f32, name="xt")
                    nc.sync.dma_start(xt[:, 0:glen], x3[:, ko, ts0 : ts0 + glen])
                    xts.append(xt)
                    # square on DVE keeps the Scalar engine free for Signs
                    xsq = xsqp.tile([P, TG], fp16, name="xsq")
                    nc.vector.tensor_tensor(
                        xsq[:, 0:glen], xt[:, 0:glen], xt[:, 0:glen], op=ALU.mult
                    )
                    nc.tensor.matmul(
                        ps_ss[:, 0:glen], ones_t[:], xsq[:, 0:glen],
                        start=(ko == 0), stop=(ko == KO - 1),
                    )
                # inv = 1/sqrt(ss/D + eps), broadcast over partitions already
                rms = stats.tile([P, TG], f32, name="rms")
                nc.scalar.activation(
                    rms[:, 0:glen], ps_ss[:, 0:glen], AF.Sqrt,
                    scale=1.0 / D_IN, bias=eps_t[:, 0:1],
                )
                inv = stats.tile([P, TG], f32, name="inv")
                nc.vector.reciprocal(inv[:, 0:glen], rms[:, 0:glen])
                if uniform_gamma:
                    # gamma folded into tau/c8 on host: one scaled inv per dtype
                    giv16 = givp.tile([P, TG], f32, name="giv16")
                    nc.vector.tensor_scalar(
                        giv16[:, 0:glen], inv[:, 0:glen], tau_sb[:, 0:1], None,
                        op0=ALU.mult,
                    )
                    giv8 = None
                    if B8 > 0:
                        giv8 = givp.tile([P, TG], f32, name="giv8")
                        nc.vector.tensor_scalar(
                            giv8[:, 0:glen], inv[:, 0:glen], c8_sb[:, 0:1], None,
                            op0=ALU.mult,
                        )
                for ko in range(KO):
                    # xnT = xt * (inv * gs[k])
                    if ko < A16:
                        dst = xnT16[:, ko, ts0 : ts0 + glen]
                        gsc = gs16[:, ko : ko + 1]
                        giv = giv16 if uniform_gamma else None
                    else:
                        g8i = (ko - A16) // 2
                        pl = (ko - A16) % 2
                        dst = xnT8[g8i][:, pl, ts0 : ts0 + glen]
                        gsc = gs8[:, ko : ko + 1]
                        giv = giv8 if uniform_gamma else None
                    if giv is None:
                        giv = givp.tile([P, TG], f32, name="giv")
                        nc.vector.tensor_scalar(
                            giv[:, 0:glen], inv[:, 0:glen], gsc, None, op0=ALU.mult
                        )
                    nc.vector.tensor_tensor(
                        dst, xts[ko][:, 0:glen], giv[:, 0:glen], op=ALU.mult
                    )

            # ---- weight quantization for one o-block ----
            def quantize_ob(ob):
                osl = slice(ob * OB, (ob + 1) * OB)
                wq16 = (
                    wq16p.tile([P, A16, OB], fp16, name="wq16")
                    if A16 > 0
                    else None
                )
                wq8s = [
                    wq8p.tile([P, 2, OB], fp8, name=f"wq8_{g}") for g in range(B8)
                ]
                for ko in range(KO):
                    wr = wrawp.tile([P, OB], f32)
                    nc.sync.dma_start(wr[:], w3[:, ko, osl])
                    # 2*ternary = sign(w - tau) + sign(w + tau) in {-2, 0, 2}
                    m1 = wmp.tile([P, OB], fp16)
                    nc.scalar.activation(m1[:], wr[:], AF.Sign, bias=ntaub_sb[:, 0:1])
                    m2 = wmp.tile([P, OB], fp16)
                    nc.scalar.activation(m2[:], wr[:], AF.Sign, bias=taub_sb[:, 0:1])
                    if ko < A16:
                        nc.vector.tensor_tensor(
                            wq16[:, ko, :], m1[:], m2[:], op=ALU.add
                        )
                    else:
                        g8i = (ko - A16) // 2
                        pl = (ko - A16) % 2
                        tmp = wmp.tile([P, OB], fp16)
                        nc.vector.tensor_tensor(tmp[:], m1[:], m2[:], op=ALU.add)
                        nc.vector.tensor_scalar_mul(wq8s[g8i][:, pl, :], tmp[:], QP)
                return wq16, wq8s

            # ---- main matmul block for one (ob, token-tile) ----
            # ko-issue order: optionally spread the fp8 DoubleRow matmuls
            # between fp16 ones so their 256-col LDWEIGHTS can prefetch
            # during a neighboring fp16 fill instead of stalling back-to-back.
            order = [("f", ko) for ko in range(A16)] + [("d", g) for g in range(B8)]
            if int(os.environ.get("BASS_INTERLEAVE", "1")) and A16 > 0 and B8 > 0:
                order = []
                fi, di = 0, 0
                stride = max(1, (A16 + B8 - 1) // B8)
                for k in range(A16 + B8):
                    if di < B8 and (k % (stride + 1) == stride or fi >= A16):
                        order.append(("d", di)); di += 1
                    else:
                        order.append(("f", fi)); fi += 1

            def main_t(wq16, wq8s, ob, t):
                tsl = slice(t * P, (t + 1) * P)
                pss = [mps.tile([P, 512], f32, name=f"ps{ch}") for ch in range(NCH)]
                for k, (kind, idx) in enumerate(order):
                    first, last = k == 0, k == len(order) - 1
                    if kind == "f":
                        lt = xnT16[:, idx, tsl]
                        for ch in range(NCH):
                            nc.tensor.matmul(
                                pss[ch][:], lt,
                                wq16[:, idx, ch * 512 : (ch + 1) * 512],
                                start=first, stop=last,
                            )
                    else:
                        lt8 = xnT8[idx][:, :, tsl]
                        for ch in range(NCH):
                            nc.tensor.matmul(
                                pss[ch][:], lt8,
                                wq8s[idx][:, :, ch * 512 : (ch + 1) * 512],
                                start=first, stop=last,
                                perf_mode=DR,
                            )
                # psum drain on the Scalar engine; DVE handles the x-side chain
                ot = op.tile([P, OB], fp16, name="ot")
                for ch in range(NCH):
                    nc.scalar.activation(
                        ot[:, ch * 512 : (ch + 1) * 512], pss[ch][:], AF.Copy
                    )
                nc.sync.dma_start(out[tsl, ob * OB : (ob + 1) * OB], ot[:])

            # ---- emission order: pipeline phase X under the matmul stream ----
            # graduated first groups shrink the serial preamble: the first
            # token tile's RMSNorm chain covers 128 tokens, not 512.
            groups = [(0, P), (P, P), (2 * P, 2 * P)] + [
                (ts, TG) for ts in range(TG, TPC, TG)
            ]
            # token-tile t is ready after group gi(t)
            t_ready = []
            for gi, (ts, gl) in enumerate(groups):
                t_ready += [gi] * (gl // P)

            phase_x(*groups[0])
            wq_cur = quantize_ob(0)
            emitted = 1
            wq_nxt = None
            for ob in range(NOB):
                for t in range(NT):
                    if ob == 0:
                        need = t_ready[min(t + 3, NT - 1)]
                        while emitted <= need and emitted < len(groups):
                            phase_x(*groups[emitted])
                            emitted += 1
                    main_t(*wq_cur, ob, t)
                    if t == 7 and ob + 1 < NOB:
                        wq_nxt = quantize_ob(ob + 1)
                if ob + 1 < NOB:
                    wq_cur = wq_nxt

    _legalize_waits(nc)
    return nc


def _build_fast_kernel():
    """Uniform-gamma fast path: x staged fp16, per-token RMSNorm scale
    deferred to the PSUM drain (per-partition activation scale), so the
    main matmul stream depends only on the x DMA and weight quantization.

    out[t,o] = (sum_k fp16(x[t,k]) * 2*tern[k,o]
                + sum_k e4m3(32*x[t,k]) * 2^-5*2*tern[k,o]) * inv[t]*g0*s/2
    """
    import concourse.bass as bass
    import concourse.tile as tile
    from concourse import mybir
    from concourse.masks import make_identity

    f32 = mybir.dt.float32
    fp16 = mybir.dt.float16
    fp8 = mybir.dt.float8e4
    AF = mybir.ActivationFunctionType
    ALU = mybir.AluOpType
    DR = mybir.MatmulPerfMode.DoubleRow

    nc = bass.Bass()
    x_in = nc.dram_tensor("x", [D_IN, TPC], fp16, kind="ExternalInput")
    wt_in = nc.dram_tensor("wt", [D_IN, OPC], f32, kind="ExternalInput")
    # scalars = [tau_bias, oscale]: tau_bias thresholds raw weights (host-
    # nudged one ulp on exact ties); oscale = gamma0 * scale / 2.
    s_in = nc.dram_tensor("scalars", [2], f32, kind="ExternalInput")
    out = nc.dram_tensor("out", [TPC, OPC], fp16, kind="ExternalOutput")

    x3 = x_in.rearrange("(ko p) t -> p ko t", p=P)
    w3 = wt_in.rearrange("(ko p) o -> p ko o", p=P)

    with tile.TileContext(nc) as tc:
        with (
            tc.tile_pool(name="singles", bufs=1) as singles,
            tc.tile_pool(name="xt8", bufs=10) as xt8p,
            tc.tile_pool(name="xsq", bufs=3) as xsqp,
            tc.tile_pool(name="stats", bufs=2) as stats,
            tc.tile_pool(name="wraw", bufs=3) as wrawp,
            tc.tile_pool(name="wm", bufs=3) as wmp,
            tc.tile_pool(name="wq16", bufs=2) as wq16p,
            tc.tile_pool(name="wq8", bufs=2) as wq8p,
            tc.tile_pool(name="op", bufs=4) as op,
            tc.tile_pool(name="ssps", bufs=2, space="PSUM") as tps,
            tc.tile_pool(name="tpps", bufs=2, space="PSUM") as tpps,
            tc.tile_pool(name="mps", bufs=4 // NCH, space="PSUM") as mps,
        ):
            ones_t = singles.tile([P, P], fp16)
            nc.vector.memset(ones_t[:], 1.0)
            ident = singles.tile([P, P], f32)
            make_identity(nc, ident)
            eps_t = singles.tile([P, 1], f32)
            nc.vector.memset(eps_t[:], EPS_RMS)
            taub_sb = singles.tile([P, 1], f32)
            nc.sync.dma_start(taub_sb[:], s_in[0:1].to_broadcast((P, 1)))
            osc_sb = singles.tile([P, 1], f32)
            nc.sync.dma_start(osc_sb[:], s_in[1:2].to_broadcast((P, 1)))
            ntaub_sb = singles.tile([P, 1], f32)
            nc.vector.tensor_scalar_mul(ntaub_sb[:], taub_sb[:], -1.0)

            xnT16 = None
            if A16 > 0:
                xnT16 = singles.tile([P, A16, TPC], fp16, name="xnT16")
            xnT8 = [
                singles.tile([P, 2, TPC], fp8, name=f"xnT8_{g}") for g in range(B8)
            ]
            # per-token drain scale, one fp32 column per 128-token tile
            ocol = singles.tile([P, NT], f32)

            def phase_x(ts0, glen):
                ps_ss = tps.tile([P, TG], f32, name="ps_ss")
                # batched DMAs: fp16 k-tiles straight into xnT16, fp8 pairs
                # into transient fp16 tiles (squared + scaled to e4m3)
                for k0 in range(0, A16, 4):
                    kc = min(4, A16 - k0)
                    nc.sync.dma_start(
                        xnT16[:, k0 : k0 + kc, ts0 : ts0 + glen],
                        x3[:, k0 : k0 + kc, ts0 : ts0 + glen],
                    )
                xt8s = []
                for g in range(B8):
                    ko0 = A16 + 2 * g
                    xt = xt8p.tile([P, 2, TG], fp16, name="xt8")
                    nc.sync.dma_start(
                        xt[:, :, 0:glen], x3[:, ko0 : ko0 + 2, ts0 : ts0 + glen]
                    )
                    xt8s.append(xt)
                    for pl in range(2):
                        nc.vector.tensor_scalar_mul(
                            xnT8[g][:, pl, ts0 : ts0 + glen], xt[:, pl, 0:glen],
                            1.0 / QP,
                        )
                for ko in range(KO):
                    if ko < A16:
                        src = xnT16[:, ko, ts0 : ts0 + glen]
                    else:
                        src = xt8s[(ko - A16) // 2][:, (ko - A16) % 2, 0:glen]
                    xsq = xsqp.tile([P, TG], fp16, name="xsq")
                    nc.vector.tensor_tensor(xsq[:, 0:glen], src, src, op=ALU.mult)
                    nc.tensor.matmul(
                        ps_ss[:, 0:glen], ones_t[:], xsq[:, 0:glen],
                        start=(ko == 0), stop=(ko == KO - 1),
                    )
                rms = stats.tile([P, TG], f32, name="rms")
                nc.scalar.activation(
                    rms[:, 0:glen], ps_ss[:, 0:glen], AF.Sqrt,
                    scale=1.0 / D_IN, bias=eps_t[:, 0:1],
                )
                inv = stats.tile([P, TG], f32, name="inv")
                nc.vector.reciprocal(inv[:, 0:glen], rms[:, 0:glen])
                # row -> column: transpose each 128-token slice of the
                # broadcast inv, keep one column, fold in gamma0*s/2
                for i in range(glen // P):
                    t = ts0 // P + i
                    pst = tpps.tile([P, P], f32, name="pst")
                    nc.tensor.transpose(
                        pst[:], inv[:, i * P : (i + 1) * P], ident[:]
                    )
                    nc.vector.tensor_scalar(
                        ocol[:, t : t + 1], pst[:, 0:1], osc_sb[:, 0:1], None,
                        op0=ALU.mult,
                    )

            def quantize_ob(ob, dma_chunk=4):
                osl = slice(ob * OB, (ob + 1) * OB)
                wq16 = (
                    wq16p.tile([P, A16, OB], fp16, name="wq16")
                    if A16 > 0
                    else None
                )
                wq8s = [
                    wq8p.tile([P, 2, OB], fp8, name=f"wq8_{g}") for g in range(B8)
                ]
                for k0 in range(0, KO, dma_chunk):
                    kc = min(dma_chunk, KO - k0)
                    wr = wrawp.tile([P, dma_chunk, OB], f32, name="wr")
                    nc.sync.dma_start(wr[:, 0:kc, :], w3[:, k0 : k0 + kc, osl])
                    for j in range(kc):
                        ko = k0 + j
                        wrj = wr[:, j, :]
                        if ko < A16:
                            # Scalar engine quantizes the fp16 k-tiles ...
                            m1 = wmp.tile([P, OB], fp16, name="m1")
                            nc.scalar.activation(
                                m1[:], wrj, AF.Sign, bias=ntaub_sb[:, 0:1]
                            )
                            m2 = wmp.tile([P, OB], fp16, name="m2")
                            nc.scalar.activation(
                                m2[:], wrj, AF.Sign, bias=taub_sb[:, 0:1]
                            )
                            nc.vector.tensor_tensor(
                                wq16[:, ko, :], m1[:], m2[:], op=ALU.add
                            )
                        else:
                            # ... while DVE quantizes the fp8 ones in parallel:
                            # wq8 = ((w > tau)*2QP) - ((w < -tau)*2QP)
                            g8i = (ko - A16) // 2
                            pl = (ko - A16) % 2
                            mp = wmp.tile([P, OB], fp16, name="mp")
                            nc.vector.tensor_scalar(
                                mp[:], wrj, taub_sb[:, 0:1], 2.0 * QP,
                                op0=ALU.is_gt, op1=ALU.mult,
                            )
                            mn = wmp.tile([P, OB], fp16, name="mn")
                            nc.vector.tensor_scalar(
                                mn[:], wrj, ntaub_sb[:, 0:1], 2.0 * QP,
                                op0=ALU.is_lt, op1=ALU.mult,
                            )
                            nc.vector.tensor_tensor(
                                wq8s[g8i][:, pl, :], mp[:], mn[:], op=ALU.subtract
                            )
                return wq16, wq8s

            order = [("f", ko) for ko in range(A16)] + [("d", g) for g in range(B8)]
            if int(os.environ.get("BASS_INTERLEAVE", "1")) and A16 > 0 and B8 > 0:
                order = []
                fi, di = 0, 0
                stride = max(1, (A16 + B8 - 1) // B8)
                for k in range(A16 + B8):
                    if di < B8 and (k % (stride + 1) == stride or fi >= A16):
                        order.append(("d", di)); di += 1
                    else:
                        order.append(("f", fi)); fi += 1

            def main_t(wq16, wq8s, ob, t):
                tsl = slice(t * P, (t + 1) * P)
                pss = [mps.tile([P, 512], f32, name=f"ps{ch}") for ch in range(NCH)]
                for k, (kind, idx) in enumerate(order):
                    first, last = k == 0, k == len(order) - 1
                    if kind == "f":
                        lt = xnT16[:, idx, tsl]
                        for ch in range(NCH):
                            nc.tensor.matmul(
                                pss[ch][:], lt,
                                wq16[:, idx, ch * 512 : (ch + 1) * 512],
                                start=first, stop=last,
                            )
                    else:
                        lt8 = xnT8[idx][:, :, tsl]
                        for ch in range(NCH):
                            nc.tensor.matmul(
                                pss[ch][:], lt8,
                                wq8s[idx][:, :, ch * 512 : (ch + 1) * 512],
                                start=first, stop=last,
                                perf_mode=DR,
                            )
                ot = op.tile([P, OB], fp16, name="ot")
                for ch in range(NCH):
                    nc.vector.tensor_scalar(
                        ot[:, ch * 512 : (ch + 1) * 512], pss[ch][:],
                        ocol[:, t : t + 1], None, op0=ALU.mult,
                    )
                nc.sync.dma_start(out[tsl, ob * OB : (ob + 1) * OB], ot[:])

            groups = [(0, P), (P, P), (2 * P, 2 * P)] + [
                (ts, TG) for ts in range(TG, TPC, TG)
            ]
            t_ready = []
            for gi, (ts, gl) in enumerate(groups):
                t_ready += [gi] * (gl // P)

            phase_x(*groups[0])
            wq_cur = quantize_ob(0)
            emitted = 1
            wq_nxt = None
            for ob in range(NOB):
                for t in range(NT):
                    if ob == 0:
                        need = t_ready[min(t + 3, NT - 1)]
                        while emitted <= need and emitted < len(groups):
                            phase_x(*groups[emitted])
                            emitted += 1
                    main_t(wq_cur[0], wq_cur[1], ob, t)
                    if t == 7 and ob + 1 < NOB:
                        wq_nxt = quantize_ob(ob + 1)
                if ob + 1 < NOB:
                    wq_cur = wq_nxt

    _legalize_waits(nc)
    return nc


def _ensure_ntff_hook():
    """Provide antenv.axon_hooks (missing from this image) so that
    run_bass_kernel_spmd(trace=True) can reach the libaxon NTFF profiler."""
    import types

    try:
        from antenv.axon_hooks import get_axon_ntff_profile_hook  # noqa: F401

        return True
    except ImportError:
        pass
    try:
        import antenv
        from trn_agent_boot.trn_boot import _ntff_profile_via_ctypes

        hook = _ntff_profile_via_ctypes("/opt/axon/libaxon_pjrt.so")
        mod = types.ModuleType("antenv.axon_hooks")
        _state = {"hook": hook}
        mod.set_axon_ntff_profile_hook = lambda h: _state.__setitem__("hook", h)
        mod.get_axon_ntff_profile_hook = lambda: _state["hook"]
        sys.modules["antenv.axon_hooks"] = mod
        antenv.axon_hooks = mod
        return hook is not None
    except Exception:
        return False


def _run(nc, in_maps, trace, tag):
    from concourse.bass_utils import run_bass_kernel_spmd

    kwargs = {}
    if trace and _ensure_ntff_hook():
        kwargs = dict(trace=True, trace_cores=list(range(N_CORES)))
        base = os.environ.get("BASS_PROBLEM_TRACE_DIR")
        if base:
            tdir = os.path.join(base, tag)
            os.makedirs(tdir, exist_ok=True)
            kwargs["tmpdir"] = tdir
    try:
        res = run_bass_kernel_spmd(nc, in_maps, list(range(N_CORES)), **kwargs)
    except Exception:
        if not kwargs:
            raise
        # tracing path failed; fall back to a plain run
        res = run_bass_kernel_spmd(nc, in_maps, list(range(N_CORES)))
    if trace:
        LAST_PROFILE[tag] = {
            "exec_time_ns": res.exec_time_ns,
            "mean_exec_time_ns": res.mean_exec_time_ns,
        }
    return res.results


def kernel(x, weight, gamma):
    trace = bool(int(os.environ.get("BASS_PROBLEM_TRACE", "0")))

    x = np.ascontiguousarray(np.asarray(x, dtype=np.float32))
    weight = np.ascontiguousarray(np.asarray(weight, dtype=np.float32))
    gamma = np.ascontiguousarray(np.asarray(gamma, dtype=np.float32))
    assert x.shape == (B, S, D_IN) and weight.shape == (D_OUT, D_IN)

    uniform_gamma = bool(np.all(gamma == gamma[0]))
    bkey = ("k2", uniform_gamma)
    if bkey not in _BUILT:
        _BUILT[bkey] = (
            _build_fast_kernel()
            if uniform_gamma
            else _build_main_kernel(uniform_gamma=False)
        )

    # --- host scalar preprocessing: global scale + Sign-tie nudge ---
    scale = np.float32(max(np.abs(weight).mean(dtype=np.float64), EPS_SCALE))
    tau = np.float32(0.5) * scale
    # Sign(w -+ tau_b) returns 0 on an exact tie, which would quantize that
    # weight to half a quantum.  Reference round-half-even maps |w| == tau to
    # 0, and |w| strictly between tau and nextafter(tau) cannot exist in
    # fp32, so nudging the bias one ulp up when a tie exists is exact.
    tau_b = tau
    aw = np.abs(weight)
    if (aw == tau_b).any():
        tau_b = np.nextafter(tau, np.float32(np.inf), dtype=np.float32)
        if (aw == tau_b).any():
            tau_b = tau
    del aw
    # --- stage host-transposed shards ---
    xT = np.ascontiguousarray(x.reshape(T, D_IN).T)        # [D_IN, T]
    wT = np.ascontiguousarray(weight.T)                    # [D_IN, D_OUT]
    wts = [np.ascontiguousarray(wT[:, j * OPC : (j + 1) * OPC]) for j in range(H)]
    if uniform_gamma:
        g0 = np.float32(gamma[0])
        oscale = np.float32(np.float32(g0 * scale) / np.float32(2.0))
        scalars = np.array([tau_b, oscale], dtype=np.float32)
        xts = [
            np.ascontiguousarray(xT[:, tg * TPC : (tg + 1) * TPC]).astype(np.float16)
            for tg in range(G)
        ]
        in2 = [
            {"x": xts[c // H], "wt": wts[c % H], "scalars": scalars}
            for c in range(N_CORES)
        ]
    else:
        c8 = np.float32(scale / np.float32(2.0 * QP))
        scalars = np.array([tau, tau_b, c8], dtype=np.float32)
        xts = [
            np.ascontiguousarray(xT[:, tg * TPC : (tg + 1) * TPC]) for tg in range(G)
        ]
        in2 = [
            {
                "x": xts[c // H],
                "wt": wts[c % H],
                "gamma": gamma,
                "scalars": scalars,
            }
            for c in range(N_CORES)
        ]
    res2 = _run(_BUILT[bkey], in2, trace, "k2")
    out = np.empty((T, D_OUT), dtype=np.float32)
    for c in range(N_CORES):
        tg, j = c // H, c % H
        out[tg * TPC : (tg + 1) * TPC, j * OPC : (j + 1) * OPC] = res2[c]["out"]
    return out.reshape(B, S, D_OUT)


# revision 46
# speedup vs baseline: 1.0249x; 1.0249x over previous
"""BitLinear (RMSNorm + ternary-quantized matmul) TRN2 kernel, v2.

Computation (reference semantics):
    x_norm = x * rsqrt(mean(x^2, -1) + 1e-6) * gamma          [B,S,Din]
    scale  = max(mean(|weight|), 1e-5)                        scalar
    wq     = round(clip(weight/scale, -1, 1))  in {-1,0,1}    [Dout,Din]
    out    = (x_norm @ wq.T) * scale                          [B,S,Dout]

Distribution (8 cores, full inputs in / full output out): 2-D shard,
4 token-groups x 2 output-feature halves.  Core c handles tokens
[tg*2048,(tg+1)*2048) x out-features [j*4096,(j+1)*4096), tg=c//2, j=c%2.
x is staged host-transposed ([Din, T] slices) so no PE transposes are
needed on device; per-token sum(x^2) is computed with an all-ones
stationary matmul that leaves the result broadcast across partitions,
exactly the layout needed to scale x^T columns.

Contraction split (accuracy/speed trade): the first A16 of 16 k-tiles
(128 each) run as fp16 matmuls (1 col/cycle); the last 2*B8 k-tiles run
as fp8e4 DoubleRow matmuls (2 k-planes per instruction, 2 MACs/cell).
Ternary weights are exact in both dtypes; only the fp8 cast of x_norm
loses precision (measured L2 rel err 1.2e-2/1.46e-2/1.69e-2 for
B8=2/3/4 vs the 2e-2 gate; fp16-only is 7.7e-4).

Scale factoring keeps every fp8 operand in e4m3 normal range with an
exactly representable weight value:
    fp16 part: xnT = fp16(xn * gamma * tau), w = {-2,0,2}      (tau=s/2)
    fp8  part: xq8 = e4m3(xn * gamma * 16 s), w8 = {-2,0,2} * 2^-5
    both products equal xn * gamma * s * ternary, one PSUM group.

The global scale s = max(mean|w|, 1e-5) and the Sign-tie nudge are
computed on host (pure scalar preprocessing); output is staged fp16 on
device and assembled/upcast to fp32 on host.
"""

import os
import sys

sys.path.insert(0, "/opt/trn_rl_repo")

import numpy as np

N_CORES = 8
B, S, D_IN, D_OUT = 4, 2048, 2048, 8192
T = B * S                    # 8192 tokens
G, H = 4, 2                  # token groups x out-feature shards
TPC = T // G                 # 2048 tokens per core
OPC = D_OUT // H             # 4096 out features per core
P = 128
KO = D_IN // P               # 16 k-tiles of 128
A16 = int(os.environ.get("BASS_A16", "8"))   # fp16 k-tiles (rest run fp8-DR)
B8 = (KO - A16) // 2         # fp8 DoubleRow groups (2 k-tiles each)
TG = 512                     # tokens per RMSNorm group
NTG = TPC // TG              # 4
NT = TPC // P                # 16 token tiles per core
OB = 512                     # out-feature block (OB//512 PSUM banks)
NCH = OB // 512              # PSUM chunks per block
NOB = OPC // OB
QP = 2.0 ** -5               # fp8 weight magnitude quantum / 2
EPS_RMS = 1e-6
EPS_SCALE = 1e-5

_BUILT = {}
LAST_PROFILE = {}


def _legalize_waits(nc):
    """Split multi-wait sync_info into preceding single-wait NOPs.

    The walrus build in this container caps embedded sync waits at 1 per
    instruction (2 for EventSemaphore); Tile's kernel-tail drain exceeds it.
    """
    from concourse import mybir

    n_fixed = 0
    for bb in nc.main_func.blocks:
        out = []
        changed = False
        for inst in bb.instructions:
            si = inst.sync_info
            waits = list(si.on_wait) if si is not None and si.on_wait else []
            cap = 2 if isinstance(inst, mybir.InstEventSemaphore) else 1
            if len(waits) > cap:
                for w in waits[:-cap]:
                    out.append(
                        mybir.InstNoOp(
                            name=f"{inst.name}-ws{n_fixed}",
                            engine=inst.engine,
                            sync_info=mybir.SyncInfo(on_wait=[w], on_update=[]),
                            text_hint="waitsplit",
                            bass_nofuse=True,
                        )
                    )
                    n_fixed += 1
                si.on_wait = waits[-cap:]
                changed = True
            out.append(inst)
        if changed:
            bb.instructions = out
    return n_fixed


def _build_main_kernel(uniform_gamma=False):
    import concourse.bass as bass
    import concourse.tile as tile
    from concourse import mybir

    f32 = mybir.dt.float32
    fp16 = mybir.dt.float16
    fp8 = mybir.dt.float8e4
    AF = mybir.ActivationFunctionType
    ALU = mybir.AluOpType
    DR = mybir.MatmulPerfMode.DoubleRow

    nc = bass.Bass()
    x_in = nc.dram_tensor("x", [D_IN, TPC], f32, kind="ExternalInput")
    wt_in = nc.dram_tensor("wt", [D_IN, OPC], f32, kind="ExternalInput")
    g_in = nc.dram_tensor("gamma", [D_IN], f32, kind="ExternalInput")
    # scalars = [tau, tau_bias, c8]: tau = scale/2; tau_bias is tau possibly
    # nudged one ulp up by the host so no |w| bit-equals it (Sign(0) at an
    # exact tie would emit a half-quantum); c8 = 16*scale (fp8 x prescale).
    s_in = nc.dram_tensor("scalars", [3], f32, kind="ExternalInput")
    out = nc.dram_tensor("out", [TPC, OPC], fp16, kind="ExternalOutput")

    x3 = x_in.rearrange("(ko p) t -> p ko t", p=P)    # [128, 16, TPC]
    w3 = wt_in.rearrange("(ko p) o -> p ko o", p=P)   # [128, 16, OPC]

    with tile.TileContext(nc) as tc:
        with (
            tc.tile_pool(name="singles", bufs=1) as singles,
            tc.tile_pool(name="xt", bufs=18) as xtp,
            tc.tile_pool(name="xsq", bufs=3) as xsqp,
            tc.tile_pool(name="stats", bufs=2) as stats,
            tc.tile_pool(name="giv", bufs=3) as givp,
            tc.tile_pool(name="wraw", bufs=3) as wrawp,
            tc.tile_pool(name="wm", bufs=3) as wmp,
            tc.tile_pool(name="wq16", bufs=2) as wq16p,
            tc.tile_pool(name="wq8", bufs=2) as wq8p,
            tc.tile_pool(name="op", bufs=4) as op,
            tc.tile_pool(name="ssps", bufs=2, space="PSUM") as tps,
            tc.tile_pool(name="mps", bufs=4 // NCH, space="PSUM") as mps,
        ):
            # ---- constants ----
            ones_t = singles.tile([P, P], fp16)
            nc.vector.memset(ones_t[:], 1.0)
            eps_t = singles.tile([P, 1], f32)
            nc.vector.memset(eps_t[:], EPS_RMS)
            tau_sb = singles.tile([P, 1], f32)
            nc.sync.dma_start(tau_sb[:], s_in[0:1].to_broadcast((P, 1)))
            taub_sb = singles.tile([P, 1], f32)
            nc.sync.dma_start(taub_sb[:], s_in[1:2].to_broadcast((P, 1)))
            c8_sb = singles.tile([P, 1], f32)
            nc.sync.dma_start(c8_sb[:], s_in[2:3].to_broadcast((P, 1)))
            ntaub_sb = singles.tile([P, 1], f32)
            nc.vector.tensor_scalar_mul(ntaub_sb[:], taub_sb[:], -1.0)
            gamma_sb = singles.tile([P, KO], f32)
            nc.sync.dma_start(gamma_sb[:], g_in.rearrange("(ko p) -> p ko", p=P))
            # per-(k-partition) factors folded into x^T:
            #   fp16 tiles: gamma * tau ;  fp8 tiles: gamma * c8
            gs16 = singles.tile([P, KO], f32)
            nc.vector.tensor_scalar_mul(gs16[:], gamma_sb[:], tau_sb[:, 0:1])
            gs8 = singles.tile([P, KO], f32)
            nc.vector.tensor_scalar_mul(gs8[:], gamma_sb[:], c8_sb[:, 0:1])

            # x_norm^T resident for the whole kernel
            xnT16 = None
            if A16 > 0:
                xnT16 = singles.tile([P, A16, TPC], fp16, name="xnT16")
            xnT8 = [
                singles.tile([P, 2, TPC], fp8, name=f"xnT8_{g}") for g in range(B8)
            ]

            # ---- phase X: RMSNorm for tokens [ts0, ts0+glen) via ones-matmul ----
            def phase_x(ts0, glen):
                xts = []
                ps_ss = tps.tile([P, TG], f32, name="ps_ss")
                for ko in range(KO):
                    xt = xtp.tile([P, TG], f32, name="xt")
                    nc.sync.dma_start(xt[:, 0:glen], x3[:, ko, ts0 : ts0 + glen])
                    xts.append(xt)
                    # square on DVE keeps the Scalar engine free for Signs
                    xsq = xsqp.tile([P, TG], fp16, name="xsq")
                    nc.vector.tensor_tensor(
                        xsq[:, 0:glen], xt[:, 0:glen], xt[:, 0:glen], op=ALU.mult
                    )
                    nc.tensor.matmul(
                        ps_ss[:, 0:glen], ones_t[:], xsq[:, 0:glen],
                        start=(ko == 0), stop=(ko == KO - 1),
                    )
                # inv = 1/sqrt(ss/D + eps), broadcast over partitions already
                rms = stats.tile([P, TG], f32, name="rms")
                nc.scalar.activation(
                    rms[:, 0:glen], ps_ss[:, 0:glen], AF.Sqrt,
                    scale=1.0 / D_IN, bias=eps_t[:, 0:1],
                )
                inv = stats.tile([P, TG], f32, name="inv")
                nc.vector.reciprocal(inv[:, 0:glen], rms[:, 0:glen])
                if uniform_gamma:
                    # gamma folded into tau/c8 on host: one scaled inv per dtype
                    giv16 = givp.tile([P, TG], f32, name="giv16")
                    nc.vector.tensor_scalar(
                        giv16[:, 0:glen], inv[:, 0:glen], tau_sb[:, 0:1], None,
                        op0=ALU.mult,
                    )
                    giv8 = None
                    if B8 > 0:
                        giv8 = givp.tile([P, TG], f32, name="giv8")
                        nc.vector.tensor_scalar(
                            giv8[:, 0:glen], inv[:, 0:glen], c8_sb[:, 0:1], None,
                            op0=ALU.mult,
                        )
                for ko in range(KO):
                    # xnT = xt * (inv * gs[k])
                    if ko < A16:
                        dst = xnT16[:, ko, ts0 : ts0 + glen]
                        gsc = gs16[:, ko : ko + 1]
                        giv = giv16 if uniform_gamma else None
                    else:
                        g8i = (ko - A16) // 2
                        pl = (ko - A16) % 2
                        dst = xnT8[g8i][:, pl, ts0 : ts0 + glen]
                        gsc = gs8[:, ko : ko + 1]
                        giv = giv8 if uniform_gamma else None
                    if giv is None:
                        giv = givp.tile([P, TG], f32, name="giv")
                        nc.vector.tensor_scalar(
                            giv[:, 0:glen], inv[:, 0:glen], gsc, None, op0=ALU.mult
                        )
                    nc.vector.tensor_tensor(
                        dst, xts[ko][:, 0:glen], giv[:, 0:glen], op=ALU.mult
                    )

            # ---- weight quantization for one o-block ----
            def quantize_ob(ob):
                osl = slice(ob * OB, (ob + 1) * OB)
                wq16 = (
                    wq16p.tile([P, A16, OB], fp16, name="wq16")
                    if A16 > 0
                    else None
                )
                wq8s = [
                    wq8p.tile([P, 2, OB], fp8, name=f"wq8_{g}") for g in range(B8)
                ]
                for ko in range(KO):
                    wr = wrawp.tile([P, OB], f32)
                    nc.sync.dma_start(wr[:], w3[:, ko, osl])
                    # 2*ternary = sign(w - tau) + sign(w + tau) in {-2, 0, 2}
                    m1 = wmp.tile([P, OB], fp16)
                    nc.scalar.activation(m1[:], wr[:], AF.Sign, bias=ntaub_sb[:, 0:1])
                    m2 = wmp.tile([P, OB], fp16)
                    nc.scalar.activation(m2[:], wr[:], AF.Sign, bias=taub_sb[:, 0:1])
                    if ko < A16:
                        nc.vector.tensor_tensor(
                            wq16[:, ko, :], m1[:], m2[:], op=ALU.add
                        )
                    else:
                        g8i = (ko - A16) // 2
                        pl = (ko - A16) % 2
                        tmp = wmp.tile([P, OB], fp16)
                        nc.vector.tensor_tensor(tmp[:], m1[:], m2[:], op=ALU.add)
                        nc.vector.tensor_scalar_mul(wq8s[g8i][:, pl, :], tmp[:], QP)
                return wq16, wq8s

            # ---- main matmul block for one (ob, token-tile) ----
            # ko-issue order: optionally spread the fp8 DoubleRow matmuls
            # between fp16 ones so their 256-col LDWEIGHTS can prefetch
            # during a neighboring fp16 fill instead of stalling back-to-back.
            order = [("f", ko) for ko in range(A16)] + [("d", g) for g in range(B8)]
            if int(os.environ.get("BASS_INTERLEAVE", "1")) and A16 > 0 and B8 > 0:
                order = []
                fi, di = 0, 0
                stride = max(1, (A16 + B8 - 1) // B8)
                for k in range(A16 + B8):
                    if di < B8 and (k % (stride + 1) == stride or fi >= A16):
                        order.append(("d", di)); di += 1
                    else:
                        order.append(("f", fi)); fi += 1

            def main_t(wq16, wq8s, ob, t):
                tsl = slice(t * P, (t + 1) * P)
                pss = [mps.tile([P, 512], f32, name=f"ps{ch}") for ch in range(NCH)]
                for k, (kind, idx) in enumerate(order):
                    first, last = k == 0, k == len(order) - 1
                    if kind == "f":
                        lt = xnT16[:, idx, tsl]
                        for ch in range(NCH):
                            nc.tensor.matmul(
                                pss[ch][:], lt,
                                wq16[:, idx, ch * 512 : (ch + 1) * 512],
                                start=first, stop=last,
                            )
                    else:
                        lt8 = xnT8[idx][:, :, tsl]
                        for ch in range(NCH):
                            nc.tensor.matmul(
                                pss[ch][:], lt8,
                                wq8s[idx][:, :, ch * 512 : (ch + 1) * 512],
                                start=first, stop=last,
                                perf_mode=DR,
                            )
                # psum drain on the Scalar engine; DVE handles the x-side chain
                ot = op.tile([P, OB], fp16, name="ot")
                for ch in range(NCH):
                    nc.scalar.activation(
                        ot[:, ch * 512 : (ch + 1) * 512], pss[ch][:], AF.Copy
                    )
                nc.sync.dma_start(out[tsl, ob * OB : (ob + 1) * OB], ot[:])

            # ---- emission order: pipeline phase X under the matmul stream ----
            # graduated first groups shrink the serial preamble: the first
            # token tile's RMSNorm chain covers 128 tokens, not 512.
            groups = [(0, P), (P, P), (2 * P, 2 * P)] + [
                (ts, TG) for ts in range(TG, TPC, TG)
            ]
            # token-tile t is ready after group gi(t)
            t_ready = []
            for gi, (ts, gl) in enumerate(groups):
                t_ready += [gi] * (gl // P)

            phase_x(*groups[0])
            wq_cur = quantize_ob(0)
            emitted = 1
            wq_nxt = None
            for ob in range(NOB):
                for t in range(NT):
                    if ob == 0:
                        need = t_ready[min(t + 3, NT - 1)]
                        while emitted <= need and emitted < len(groups):
                            phase_x(*groups[emitted])
                            emitted += 1
                    main_t(*wq_cur, ob, t)
                    if t == 7 and ob + 1 < NOB:
                        wq_nxt = quantize_ob(ob + 1)
                if ob + 1 < NOB:
                    wq_cur = wq_nxt

    _legalize_waits(nc)
    return nc


def _build_fast_kernel():
    """Uniform-gamma fast path: x staged fp16, per-token RMSNorm scale
    deferred to the PSUM drain (per-partition activation scale), so the
    main matmul stream depends only on the x DMA and weight quantization.

    out[t,o] = (sum_k fp16(x[t,k]) * 2*tern[k,o]
                + sum_k e4m3(32*x[t,k]) * 2^-5*2*tern[k,o]) * inv[t]*g0*s/2
    """
    import concourse.bass as bass
    import concourse.tile as tile
    from concourse import mybir
    from concourse.masks import make_identity

    f32 = mybir.dt.float32
    fp16 = mybir.dt.float16
    fp8 = mybir.dt.float8e4
    AF = mybir.ActivationFunctionType
    ALU = mybir.AluOpType
    DR = mybir.MatmulPerfMode.DoubleRow

    nc = bass.Bass()
    x_in = nc.dram_tensor("x", [D_IN, TPC], fp16, kind="ExternalInput")
    wt_in = nc.dram_tensor("wt", [D_IN, OPC], f32, kind="ExternalInput")
    # scalars = [tau_bias, oscale]: tau_bias thresholds raw weights (host-
    # nudged one ulp on exact ties); oscale = gamma0 * scale / 2.
    s_in = nc.dram_tensor("scalars", [2], f32, kind="ExternalInput")
    out = nc.dram_tensor("out", [TPC, OPC], fp16, kind="ExternalOutput")

    x3 = x_in.rearrange("(ko p) t -> p ko t", p=P)
    w3 = wt_in.rearrange("(ko p) o -> p ko o", p=P)

    with tile.TileContext(nc) as tc:
        with (
            tc.tile_pool(name="singles", bufs=1) as singles,
            tc.tile_pool(name="xt8", bufs=10) as xt8p,
            tc.tile_pool(name="xsq", bufs=3) as xsqp,
            tc.tile_pool(name="stats", bufs=2) as stats,
            tc.tile_pool(name="wraw", bufs=3) as wrawp,
            tc.tile_pool(name="wm", bufs=3) as wmp,
            tc.tile_pool(name="wq16", bufs=2) as wq16p,
            tc.tile_pool(name="wq8", bufs=2) as wq8p,
            tc.tile_pool(name="op", bufs=4) as op,
            tc.tile_pool(name="ssps", bufs=2, space="PSUM") as tps,
            tc.tile_pool(name="tpps", bufs=2, space="PSUM") as tpps,
            tc.tile_pool(name="mps", bufs=4 // NCH, space="PSUM") as mps,
        ):
            ones_t = singles.tile([P, P], fp16)
            nc.vector.memset(ones_t[:], 1.0)
            ident = singles.tile([P, P], f32)
            make_identity(nc, ident)
            eps_t = singles.tile([P, 1], f32)
            nc.vector.memset(eps_t[:], EPS_RMS)
            taub_sb = singles.tile([P, 1], f32)
            nc.sync.dma_start(taub_sb[:], s_in[0:1].to_broadcast((P, 1)))
            osc_sb = singles.tile([P, 1], f32)
            nc.sync.dma_start(osc_sb[:], s_in[1:2].to_broadcast((P, 1)))
            ntaub_sb = singles.tile([P, 1], f32)
            nc.vector.tensor_scalar_mul(ntaub_sb[:], taub_sb[:], -1.0)

            xnT16 = None
            if A16 > 0:
                xnT16 = singles.tile([P, A16, TPC], fp16, name="xnT16")
            xnT8 = [
                singles.tile([P, 2, TPC], fp8, name=f"xnT8_{g}") for g in range(B8)
            ]
            # per-token drain scale, one fp32 column per 128-token tile
            ocol = singles.tile([P, NT], f32)

            def phase_x(ts0, glen):
                ps_ss = tps.tile([P, TG], f32, name="ps_ss")
                # batched DMAs: fp16 k-tiles straight into xnT16, fp8 pairs
                # into transient fp16 tiles (squared + scaled to e4m3)
                for k0 in range(0, A16, 4):
                    kc = min(4, A16 - k0)
                    nc.sync.dma_start(
                        xnT16[:, k0 : k0 + kc, ts0 : ts0 + glen],
                        x3[:, k0 : k0 + kc, ts0 : ts0 + glen],
                    )
                xt8s = []
                for g in range(B8):
                    ko0 = A16 + 2 * g
                    xt = xt8p.tile([P, 2, TG], fp16, name="xt8")
                    nc.sync.dma_start(
                        xt[:, :, 0:glen], x3[:, ko0 : ko0 + 2, ts0 : ts0 + glen]
                    )
                    xt8s.append(xt)
                    for pl in range(2):
                        nc.vector.tensor_scalar_mul(
                            xnT8[g][:, pl, ts0 : ts0 + glen], xt[:, pl, 0:glen],
                            1.0 / QP,
                        )
                for ko in range(KO):
                    if ko < A16:
                        src = xnT16[:, ko, ts0 : ts0 + glen]
                    else:
                        src = xt8s[(ko - A16) // 2][:, (ko - A16) % 2, 0:glen]
                    xsq = xsqp.tile([P, TG], fp16, name="xsq")
                    nc.vector.tensor_tensor(xsq[:, 0:glen], src, src, op=ALU.mult)
                    nc.tensor.matmul(
                        ps_ss[:, 0:glen], ones_t[:], xsq[:, 0:glen],
                        start=(ko == 0), stop=(ko == KO - 1),
                    )
                rms = stats.tile([P, TG], f32, name="rms")
                nc.scalar.activation(
                    rms[:, 0:glen], ps_ss[:, 0:glen], AF.Sqrt,
                    scale=1.0 / D_IN, bias=eps_t[:, 0:1],
                )
                inv = stats.tile([P, TG], f32, name="inv")
                nc.vector.reciprocal(inv[:, 0:glen], rms[:, 0:glen])
                # row -> column: transpose each 128-token slice of the
                # broadcast inv, keep one column, fold in gamma0*s/2
                for i in range(glen // P):
                    t = ts0 // P + i
                    pst = tpps.tile([P, P], f32, name="pst")
                    nc.tensor.transpose(
                        pst[:], inv[:, i * P : (i + 1) * P], ident[:]
                    )
                    nc.vector.tensor_scalar(
                        ocol[:, t : t + 1], pst[:, 0:1], osc_sb[:, 0:1], None,
                        op0=ALU.mult,
                    )

            def quantize_ob(ob, dma_chunk=4):
                osl = slice(ob * OB, (ob + 1) * OB)
                wq16 = (
                    wq16p.tile([P, A16, OB], fp16, name="wq16")
                    if A16 > 0
                    else None
                )
                wq8s = [
                    wq8p.tile([P, 2, OB], fp8, name=f"wq8_{g}") for g in range(B8)
                ]
                for k0 in range(0, KO, dma_chunk):
                    kc = min(dma_chunk, KO - k0)
                    wr = wrawp.tile([P, dma_chunk, OB], f32, name="wr")
                    nc.sync.dma_start(wr[:, 0:kc, :], w3[:, k0 : k0 + kc, osl])
                    for j in range(kc):
                        ko = k0 + j
                        wrj = wr[:, j, :]
                        m1 = wmp.tile([P, OB], fp16, name="m1")
                        nc.scalar.activation(
                            m1[:], wrj, AF.Sign, bias=ntaub_sb[:, 0:1]
                        )
                        m2 = wmp.tile([P, OB], fp16, name="m2")
                        nc.scalar.activation(
                            m2[:], wrj, AF.Sign, bias=taub_sb[:, 0:1]
                        )
                        if ko < A16:
                            nc.vector.tensor_tensor(
                                wq16[:, ko, :], m1[:], m2[:], op=ALU.add
                            )
                        else:
                            g8i = (ko - A16) // 2
                            pl = (ko - A16) % 2
                            tmp = wmp.tile([P, OB], fp16, name="tmp")
                            nc.vector.tensor_tensor(tmp[:], m1[:], m2[:], op=ALU.add)
                            nc.vector.tensor_scalar_mul(
                                wq8s[g8i][:, pl, :], tmp[:], QP
                            )
                return wq16, wq8s

            order = [("f", ko) for ko in range(A16)] + [("d", g) for g in range(B8)]
            if int(os.environ.get("BASS_INTERLEAVE", "1")) and A16 > 0 and B8 > 0:
                order = []
                fi, di = 0, 0
                stride = max(1, (A16 + B8 - 1) // B8)
                for k in range(A16 + B8):
                    if di < B8 and (k % (stride + 1) == stride or fi >= A16):
                        order.append(("d", di)); di += 1
                    else:
                        order.append(("f", fi)); fi += 1

            def main_t(wq16, wq8s, ob, t):
                tsl = slice(t * P, (t + 1) * P)
                pss = [mps.tile([P, 512], f32, name=f"ps{ch}") for ch in range(NCH)]
                for k, (kind, idx) in enumerate(order):
                    first, last = k == 0, k == len(order) - 1
                    if kind == "f":
                        lt = xnT16[:, idx, tsl]
                        for ch in range(NCH):
                            nc.tensor.matmul(
                                pss[ch][:], lt,
                                wq16[:, idx, ch * 512 : (ch + 1) * 512],
                                start=first, stop=last,
                            )
                    else:
                        lt8 = xnT8[idx][:, :, tsl]
                        for ch in range(NCH):
                            nc.tensor.matmul(
                                pss[ch][:], lt8,
                                wq8s[idx][:, :, ch * 512 : (ch + 1) * 512],
                                start=first, stop=last,
                                perf_mode=DR,
                            )
                ot = op.tile([P, OB], fp16, name="ot")
                for ch in range(NCH):
                    nc.vector.tensor_scalar(
                        ot[:, ch * 512 : (ch + 1) * 512], pss[ch][:],
                        ocol[:, t : t + 1], None, op0=ALU.mult,
                    )
                nc.sync.dma_start(out[tsl, ob * OB : (ob + 1) * OB], ot[:])

            groups = [(0, P), (P, P), (2 * P, 2 * P)] + [
                (ts, TG) for ts in range(TG, TPC, TG)
            ]
            t_ready = []
            for gi, (ts, gl) in enumerate(groups):
                t_ready += [gi] * (gl // P)

            phase_x(*groups[0])
            wq_cur = quantize_ob(0)
            emitted = 1
            wq_nxt = None
            for ob in range(NOB):
                for t in range(NT):
                    if ob == 0:
                        need = t_ready[min(t + 3, NT - 1)]
                        while emitted <= need and emitted < len(groups):
                            phase_x(*groups[emitted])
                            emitted += 1
                    main_t(wq_cur[0], wq_cur[1], ob, t)
                    if t == 7 and ob + 1 < NOB:
                        wq_nxt = quantize_ob(ob + 1)
                if ob + 1 < NOB:
                    wq_cur = wq_nxt

    _legalize_waits(nc)
    return nc


def _ensure_ntff_hook():
    """Provide antenv.axon_hooks (missing from this image) so that
    run_bass_kernel_spmd(trace=True) can reach the libaxon NTFF profiler."""
    import types

    try:
        from antenv.axon_hooks import get_axon_ntff_profile_hook  # noqa: F401

        return True
    except ImportError:
        pass
    try:
        import antenv
        from trn_agent_boot.trn_boot import _ntff_profile_via_ctypes

        hook = _ntff_profile_via_ctypes("/opt/axon/libaxon_pjrt.so")
        mod = types.ModuleType("antenv.axon_hooks")
        _state = {"hook": hook}
        mod.set_axon_ntff_profile_hook = lambda h: _state.__setitem__("hook", h)
        mod.get_axon_ntff_profile_hook = lambda: _state["hook"]
        sys.modules["antenv.axon_hooks"] = mod
        antenv.axon_hooks = mod
        return hook is not None
    except Exception:
        return False


def _run(nc, in_maps, trace, tag):
    from concourse.bass_utils import run_bass_kernel_spmd

    kwargs = {}
    if trace and _ensure_ntff_hook():
        kwargs = dict(trace=True, trace_cores=list(range(N_CORES)))
        base = os.environ.get("BASS_PROBLEM_TRACE_DIR")
        if base:
            tdir = os.path.join(base, tag)
            os.makedirs(tdir, exist_ok=True)
            kwargs["tmpdir"] = tdir
    try:
        res = run_bass_kernel_spmd(nc, in_maps, list(range(N_CORES)), **kwargs)
    except Exception:
        if not kwargs:
            raise
        # tracing path failed; fall back to a plain run
        res = run_bass_kernel_spmd(nc, in_maps, list(range(N_CORES)))
    if trace:
        LAST_PROFILE[tag] = {
            "exec_time_ns": res.exec_time_ns,
            "mean_exec_time_ns": res.mean_exec_time_ns,
        }
    return res.results


def kernel(x, weight, gamma):
    trace = bool(int(os.environ.get("BASS_PROBLEM_TRACE", "0")))

    x = np.ascontiguousarray(np.asarray(x, dtype=np.float32))
    weight = np.ascontiguousarray(np.asarray(weight, dtype=np.float32))
    gamma = np.ascontiguousarray(np.asarray(gamma, dtype=np.float32))
    assert x.shape == (B, S, D_IN) and weight.shape == (D_OUT, D_IN)

    uniform_gamma = bool(np.all(gamma == gamma[0]))
    bkey = ("k2", uniform_gamma)
    if bkey not in _BUILT:
        _BUILT[bkey] = (
            _build_fast_kernel()
            if uniform_gamma
            else _build_main_kernel(uniform_gamma=False)
        )

    # --- host scalar preprocessing: global scale + Sign-tie nudge ---
    scale = np.float32(max(np.abs(weight).mean(dtype=np.float64), EPS_SCALE))
    tau = np.float32(0.5) * scale
    # Sign(w -+ tau_b) returns 0 on an exact tie, which would quantize that
    # weight to half a quantum.  Reference round-half-even maps |w| == tau to
    # 0, and |w| strictly between tau and nextafter(tau) cannot exist in
    # fp32, so nudging the bias one ulp up when a tie exists is exact.
    tau_b = tau
    aw = np.abs(weight)
    if (aw == tau_b).any():
        tau_b = np.nextafter(tau, np.float32(np.inf), dtype=np.float32)
        if (aw == tau_b).any():
            tau_b = tau
    del aw
    # --- stage host-transposed shards ---
    xT = np.ascontiguousarray(x.reshape(T, D_IN).T)        # [D_IN, T]
    wT = np.ascontiguousarray(weight.T)                    # [D_IN, D_OUT]
    wts = [np.ascontiguousarray(wT[:, j * OPC : (j + 1) * OPC]) for j in range(H)]
    if uniform_gamma:
        g0 = np.float32(gamma[0])
        oscale = np.float32(np.float32(g0 * scale) / np.float32(2.0))
        scalars = np.array([tau_b, oscale], dtype=np.float32)
        xts = [
            np.ascontiguousarray(xT[:, tg * TPC : (tg + 1) * TPC]).astype(np.float16)
            for tg in range(G)
        ]
        in2 = [
            {"x": xts[c // H], "wt": wts[c % H], "scalars": scalars}
            for c in range(N_CORES)
        ]
    else:
        c8 = np.float32(scale / np.float32(2.0 * QP))
        scalars = np.array([tau, tau_b, c8], dtype=np.float32)
        xts = [
            np.ascontiguousarray(xT[:, tg * TPC : (tg + 1) * TPC]) for tg in range(G)
        ]
        in2 = [
            {
                "x": xts[c // H],
                "wt": wts[c % H],
                "gamma": gamma,
                "scalars": scalars,
            }
            for c in range(N_CORES)
        ]
    res2 = _run(_BUILT[bkey], in2, trace, "k2")
    out = np.empty((T, D_OUT), dtype=np.float32)
    for c in range(N_CORES):
        tg, j = c // H, c % H
        out[tg * TPC : (tg + 1) * TPC, j * OPC : (j + 1) * OPC] = res2[c]["out"]
    return out.reshape(B, S, D_OUT)
